# revision 1
# baseline (speedup 1.0000x reference)
"""Bass/Tile TRN2 kernel for nn_DimensionScaledEqProp.

Data-parallel over batch: x rows sharded across 8 NeuronCores, weights
replicated. Per-core state (h) stays resident in SBUF across the 30
sequential steps. fp16 matmul operands, fp32 accumulation/state.

Self-contained: hardcodes shapes; host side does sharding, spectral-norm
sigma (tiny: 60 matvecs), weight folding/transposition, and the final
gather/transpose.
"""
import sys
import numpy as np

for _p in ("/opt/trn_rl_repo", "/root/.axon_site/_ro/trn_rl_repo"):
    if _p not in sys.path:
        sys.path.append(_p)

B, DIN, DH, DOUT = 4096, 512, 1024, 256
DFF = 4 * DH
STEPS = 30
N_CORES = 8
R = B // N_CORES  # rows per core = 512
GAMMA = 0.5 * min(1.0, float(np.sqrt(64.0 / DIN)))
LN_EPS = 1e-5

KD = DH // 128    # 8  k-tiles over DH
FD = DFF // 128   # 32 f-tiles over DFF
RD = R // 128     # 4  row-tiles per core
ID = DIN // 128   # 4  k-tiles over DIN
OD = DOUT // 128  # 2  out-tiles over DOUT
NH = DH // 512    # 2  psum halves over DH

_CACHE = {}


def _build_program(steps: int):
    import concourse.bass as bass
    import concourse.bacc as bacc
    import concourse.mybir as mybir
    from concourse import tile, masks

    f16 = mybir.dt.float16
    f32 = mybir.dt.float32
    AF = mybir.ActivationFunctionType
    OP = mybir.AluOpType

    nc = bacc.Bacc("TRN2", target_bir_lowering=False, debug=False,
                   enable_asserts=True, num_devices=N_CORES)

    xT_d = nc.dram_tensor("xT", [DIN, R], f16, kind="ExternalInput")
    ewT_d = nc.dram_tensor("ewT", [DIN, DH], f16, kind="ExternalInput")
    w1t_d = nc.dram_tensor("w1t", [DH, DFF], f16, kind="ExternalInput")
    b1e_d = nc.dram_tensor("b1e", [DFF, 1], f32, kind="ExternalInput")
    w2t_d = nc.dram_tensor("w2t", [DFF, DH], f16, kind="ExternalInput")
    vb1_d = nc.dram_tensor("vb1", [1, DH], f32, kind="ExternalInput")
    vb2_d = nc.dram_tensor("vb2", [1, DH], f32, kind="ExternalInput")
    hwt_d = nc.dram_tensor("hwt", [DH, DOUT], f16, kind="ExternalInput")
    hb_d = nc.dram_tensor("hb", [DOUT, 1], f32, kind="ExternalInput")
    outT_d = nc.dram_tensor("outT", [DOUT, R], f32, kind="ExternalOutput")

    with tile.TileContext(nc) as tc:
        with (
            tc.tile_pool(name="wp", bufs=1) as wp,
            tc.tile_pool(name="sp", bufs=1) as sp,
            tc.tile_pool(name="wk", bufs=2) as wk,
            tc.tile_pool(name="stp", bufs=2) as stp,
            tc.tile_pool(name="pst", bufs=4, space="PSUM") as pst,
            tc.tile_pool(name="ps1", bufs=2, space="PSUM") as ps1p,
            tc.tile_pool(name="ps2", bufs=2, space="PSUM") as ps2p,
        ):
            # ---- persistent weights / constants ----
            w1 = [wp.tile([128, DFF], f16, name=f"w1_{k}") for k in range(KD)]
            w2 = [wp.tile([128, DH], f16, name=f"w2_{f}") for f in range(FD)]
            hwt = [wp.tile([128, DOUT], f16, name=f"hwt_{k}") for k in range(KD)]
            b1s = wp.tile([128, FD], f32, name="b1s")
            hbs = wp.tile([128, OD], f32, name="hbs")
            ident = wp.tile([128, 128], f16, name="ident")

            # ---- persistent state ----
            h = [sp.tile([128, DH], f32, name=f"h_{r}") for r in range(RD)]
            xeg = [sp.tile([128, DH], f16, name=f"xeg_{r}") for r in range(RD)]
            hnT = [sp.tile([128, R], f16, name=f"hnT_{k}") for k in range(KD)]

            masks.make_identity(nc, ident[:])

            # ---- embed (transient pool, released before the step loop) ----
            with tc.tile_pool(name="ep", bufs=1) as ep:
                xts = [ep.tile([128, R], f16, name=f"xts_{i}")
                       for i in range(ID)]
                ewt = [ep.tile([128, DH], f16, name=f"ewt_{i}")
                       for i in range(ID)]
                bc1 = ep.tile([128, DH], f32, name="bc1")
                bc2 = ep.tile([128, DH], f32, name="bc2")
                for i in range(ID):
                    nc.sync.dma_start(
                        xts[i][:], xT_d.ap()[i * 128:(i + 1) * 128, :])
                    nc.sync.dma_start(
                        ewt[i][:], ewT_d.ap()[i * 128:(i + 1) * 128, :])
                nc.sync.dma_start(bc1[0:1, :], vb1_d.ap())
                nc.sync.dma_start(bc2[0:1, :], vb2_d.ap())
                nc.gpsimd.partition_broadcast(bc1[:], bc1[0:1, :])
                nc.gpsimd.partition_broadcast(bc2[:], bc2[0:1, :])

                # weight loads AFTER embed inputs: embed matmuls start
                # immediately; w1/w2 stream in behind them
                for k in range(KD):
                    nc.sync.dma_start(
                        w1[k][:], w1t_d.ap()[k * 128:(k + 1) * 128, :])
                nc.sync.dma_start(
                    b1s[:], b1e_d.ap().rearrange("(f p) o -> p (f o)", p=128))
                nc.sync.dma_start(
                    hbs[:], hb_d.ap().rearrange("(t p) o -> p (t o)", p=128))

                # h0 = x @ ewT + embed_b ; xeg = g*h0 + g*b2 (f16)
                for r in range(RD):
                    for half in range(NH):
                        sl = slice(half * 512, (half + 1) * 512)
                        pe = ps1p.tile([128, 512], f32, tag="ps1", name="pe")
                        for i in range(ID):
                            nc.tensor.matmul(
                                pe[:], xts[i][:, r * 128:(r + 1) * 128],
                                ewt[i][:, sl],
                                start=(i == 0), stop=(i == ID - 1))
                        nc.vector.tensor_tensor(
                            h[r][:, sl], pe[:], bc1[:, sl], op=OP.add)
                        nc.vector.scalar_tensor_tensor(
                            xeg[r][:, sl], h[r][:, sl], GAMMA, bc2[:, sl],
                            op0=OP.mult, op1=OP.add)

            # ---- initial LN stats on h0 (ACT sqrt once; hides table load) ----
            mv0 = stp.tile([128, RD * 2], f32, tag="mv", name="mv_init")
            for r in range(RD):
                st6 = stp.tile([128, 12], f32, tag="st6", name=f"st6_i_{r}")
                for c in range(2):
                    nc.vector.bn_stats(
                        st6[:, c * 6:(c + 1) * 6],
                        h[r][:, c * 512:(c + 1) * 512])
                nc.vector.bn_aggr(
                    mv0[:].rearrange("p (r x) -> p r x", x=2)[:, r], st6[:])
            mvv0 = mv0[:].rearrange("p (r x) -> p r x", x=2)
            ve0 = stp.tile([128, RD], f32, tag="ve", name="ve_init")
            nc.vector.tensor_scalar(
                ve0[:], mv0[:].rearrange("p (r x) -> p x r", x=2)[:, 1], LN_EPS, None, op0=OP.add)
            rv0 = stp.tile([128, RD], f32, tag="rv", name="rv_init")
            nc.vector.reciprocal(rv0[:], ve0[:])
            rs_prev = stp.tile([128, RD], f32, tag="rs", name="rs_init")
            nc.scalar.activation(rs_prev[:], rv0[:], AF.Sqrt)

            # ---- hidT pool reuses the embed pool space ----
            with tc.tile_pool(name="hp", bufs=1) as hp:
                hidT = [hp.tile([128, R], f16, name=f"hidT_{f}")
                        for f in range(FD)]

                def rstd_newton(y_out, y_seed, var_ap, tag_sfx, n_iter=2):
                    """y_out[128,1] = 1/sqrt(var+eps) via Newton from seed."""
                    hv = stp.tile([128, 1], f32, tag="hv",
                                  name=f"hv_{tag_sfx}")
                    nc.vector.tensor_scalar(
                        hv[:], var_ap, -0.5, -0.5 * LN_EPS,
                        op0=OP.mult, op1=OP.add)
                    y = y_seed
                    for it in range(n_iter):
                        a = stp.tile([128, 1], f32, tag="nwa",
                                     name=f"nwa_{tag_sfx}_{it}")
                        nc.vector.tensor_tensor(a[:], y, y, op=OP.mult)
                        nc.vector.tensor_scalar(
                            a[:], a[:], hv[:], 1.5, op0=OP.mult, op1=OP.add)
                        if it == n_iter - 1:
                            nc.vector.tensor_tensor(y_out, y, a[:], op=OP.mult)
                        else:
                            yn = stp.tile([128, 1], f32, tag="nwy",
                                          name=f"nwy_{tag_sfx}_{it}")
                            nc.vector.tensor_tensor(yn[:], y, a[:], op=OP.mult)
                            y = yn[:]

                # normalize h0 -> hnT for step 0 (one Newton polish on seed)
                mv_p, rs_p = mv0, rs_prev
                rs_fix = stp.tile([128, RD], f32, tag="rsf", name="rs_fix")
                for r in range(RD):
                    rstd_newton(rs_fix[:, r:r + 1], rs_prev[:, r:r + 1],
                                mv0[:, 2 * r + 1:2 * r + 2], f"i{r}", n_iter=1)
                rs_p = rs_fix

                def normalize(r, mean_ap, rs_col, sfx):
                    nmu = stp.tile([128, 1], f32, tag="nmu", name=f"nmu_{sfx}")
                    nc.vector.scalar_tensor_tensor(
                        nmu[:], mean_ap, -1.0, rs_col,
                        op0=OP.mult, op1=OP.mult)
                    hn16 = wk.tile([128, DH], f16, tag=f"hn16_{r}",
                                   name=f"hn16_{sfx}", bufs=1)
                    # two half-width ops so transposes of the low half can
                    # start before the high half is normalized
                    for half in range(NH):
                        sl = slice(half * 512, (half + 1) * 512)
                        nc.vector.tensor_scalar(
                            hn16[:, sl], h[r][:, sl], rs_col, nmu[:],
                            op0=OP.mult, op1=OP.add)
                    return hn16

                def transposes(r, hn16, sfx):
                    for k in range(KD):
                        tp = pst.tile([128, 128], f16, tag="tp",
                                      name=f"tp_{sfx}_{k}")
                        nc.tensor.transpose(
                            tp[:], hn16[:, k * 128:(k + 1) * 128], ident[:])
                        if k % 2 == 0:
                            nc.vector.tensor_copy(
                                hnT[k][:, r * 128:(r + 1) * 128], tp[:])
                        else:
                            nc.scalar.copy(
                                hnT[k][:, r * 128:(r + 1) * 128], tp[:])

                for r in range(RD):
                    hn = normalize(r, mv0[:, 2 * r:2 * r + 1],
                                   rs_p[:, r:r + 1], f"s0_{r}")
                    transposes(r, hn, f"s0_{r}")

                for s in range(steps):
                    last = (s == steps - 1)
                    # hidT = tanh(W1n' @ hnT + b1)
                    for f in range(FD):
                        p1 = ps1p.tile([128, 512], f32, tag="ps1",
                                       name=f"p1_{s}_{f}")
                        for k in range(KD):
                            nc.tensor.matmul(
                                p1[:], w1[k][:, f * 128:(f + 1) * 128],
                                hnT[k][:],
                                start=(k == 0), stop=(k == KD - 1))
                        nc.scalar.activation(
                            hidT[f][:], p1[:], AF.Tanh, bias=b1s[:, f:f + 1])

                    if s == 0:
                        # w2/hwt loads deferred past step-0 mm1 so w1 gets
                        # full DMA bandwidth at startup; w2 arrives during
                        # mm1 execution, well before mm2 needs it
                        for f_ in range(FD):
                            nc.sync.dma_start(
                                w2[f_][:],
                                w2t_d.ap()[f_ * 128:(f_ + 1) * 128, :])
                        for k_ in range(KD):
                            nc.sync.dma_start(
                                hwt[k_][:],
                                hwt_d.ap()[k_ * 128:(k_ + 1) * 128, :])

                    # per row-tile: matmul2 (+xeg seeded in PSUM), update,
                    # stats, rstd, normalize, transpose -- interleaved so PE
                    # never idles at the step boundary.
                    mv = stp.tile([128, RD * 2], f32, tag="mv",
                                  name=f"mv_{s}")
                    mvv = mv[:].rearrange("p (r x) -> p r x", x=2)
                    rs = stp.tile([128, RD], f32, tag="rs", name=f"rs_{s}")
                    hns = {}
                    for r in range(RD):
                        st6 = stp.tile([128, 12], f32, tag="st6",
                                       name=f"st6_{s}_{r}")
                        for half in range(NH):
                            sl = slice(half * 512, (half + 1) * 512)
                            p2 = ps2p.tile([128, 512], f32, tag="ps2",
                                           name=f"p2_{s}_{r}_{half}")
                            nc.tensor.matmul(
                                p2[:], ident[:], xeg[r][:, sl],
                                start=True, stop=False)
                            for f in range(FD):
                                nc.tensor.matmul(
                                    p2[:], hidT[f][:, r * 128:(r + 1) * 128],
                                    w2[f][:, sl],
                                    start=False, stop=(f == FD - 1))
                            nc.vector.scalar_tensor_tensor(
                                h[r][:, sl], h[r][:, sl], 1.0 - GAMMA, p2[:],
                                op0=OP.mult, op1=OP.add)
                            if not last:
                                # stats chunk for this half right away
                                nc.vector.bn_stats(
                                    st6[:, half * 6:(half + 1) * 6],
                                    h[r][:, sl])
                        if last:
                            # head prep inline: cast final h to fp16 so its
                            # transposes overlap the remaining matmul2 groups
                            hc16 = wk.tile([128, DH], f16, tag=f"hn16_{r}",
                                           name=f"hc16_{r}", bufs=1)
                            nc.vector.tensor_copy(hc16[:], h[r][:])
                            hns[r] = hc16
                            continue
                        nc.vector.bn_aggr(mvv[:, r], st6[:])
                        rstd_newton(rs[:, r:r + 1], rs_p[:, r:r + 1],
                                    mv[:, 2 * r + 1:2 * r + 2], f"{s}_{r}")
                        hns[r] = normalize(r, mv[:, 2 * r:2 * r + 1],
                                           rs[:, r:r + 1], f"{s}_{r}")
                    # transposes LAST: PE has cover work while the final
                    # row-tile's DVE chain drains, so it never idles
                    for r in range(RD):
                        transposes(r, hns[r], f"{s}_{r}")
                    mv_p, rs_p = mv, rs

                # ---- head: outT = head_w @ h.T + head_b ----
                # (hnT already holds final h transposed, prepped in-loop)
                for ot in range(OD):
                    po = ps1p.tile([128, 512], f32, tag="ps1", name=f"po_{ot}")
                    for k in range(KD):
                        nc.tensor.matmul(
                            po[:], hwt[k][:, ot * 128:(ot + 1) * 128],
                            hnT[k][:],
                            start=(k == 0), stop=(k == KD - 1))
                    osb = wk.tile([128, 512], f32, tag="osb",
                                  name=f"osb_{ot}", bufs=1)
                    nc.scalar.activation(
                        osb[:], po[:], AF.Identity, bias=hbs[:, ot:ot + 1])
                    nc.sync.dma_start(
                        outT_d.ap()[ot * 128:(ot + 1) * 128, :], osb[:])

    nc.compile()
    return nc


def _get_compiled(steps: int):
    key = ("prog", steps)
    if key not in _CACHE:
        from concourse.bass_interp import get_hw_module
        nc = _build_program(steps)
        nc.m = get_hw_module(nc.m)
        _CACHE[key] = nc
    return _CACHE[key]


def _spectral_sigma(W: np.ndarray) -> float:
    W = W.astype(np.float64)
    v = np.full(W.shape[1], 1.0 / np.sqrt(W.shape[1]))
    u = W @ v
    u = u / (np.linalg.norm(u) + 1e-12)
    for _ in range(15):
        u = W @ v
        u = u / (np.linalg.norm(u) + 1e-12)
        v = W.T @ u
        v = v / (np.linalg.norm(v) + 1e-12)
    return float(u @ (W @ v))


def _prep_host(inputs: dict) -> tuple[dict, list]:
    f = {k: np.asarray(v, dtype=np.float32) for k, v in inputs.items()}
    x, ew, eb = f["x"], f["embed_w"], f["embed_b"]
    W1, b1, W2, b2 = f["W1"], f["b1"], f["W2"], f["b2"]
    ln_g, ln_b = f["ln_g"], f["ln_b"]
    hw_, hb = f["head_w"], f["head_b"]

    s1 = _spectral_sigma(W1)
    s2 = _spectral_sigma(W2)
    W1n = (W1.astype(np.float64) / s1)
    W2n = (W2.astype(np.float64) / s2)
    # fold ln gain into W1, ln bias into b1
    W1eff = W1n * ln_g.astype(np.float64)[None, :]
    b1eff = (b1.astype(np.float64) + W1n @ ln_b.astype(np.float64))
    W2eff = GAMMA * W2n

    shared = {
        "ewT": np.ascontiguousarray(ew.T).astype(np.float16),
        "w1t": np.ascontiguousarray(W1eff.T).astype(np.float16),
        "b1e": b1eff.astype(np.float32).reshape(DFF, 1),
        "w2t": np.ascontiguousarray(W2eff.T).astype(np.float16),
        "vb1": eb.reshape(1, DH).astype(np.float32),
        "vb2": (GAMMA * b2).reshape(1, DH).astype(np.float32),
        "hwt": np.ascontiguousarray(hw_.T).astype(np.float16),
        "hb": hb.reshape(DOUT, 1).astype(np.float32),
    }
    in_maps = []
    for c in range(N_CORES):
        shard = x[c * R:(c + 1) * R, :]
        m = dict(shared)
        m["xT"] = np.ascontiguousarray(shard.T).astype(np.float16)
        in_maps.append(m)
    return shared, in_maps


def kernel(**inputs) -> np.ndarray:
    from concourse import bass_utils
    nc = _get_compiled(STEPS)
    _, in_maps = _prep_host(inputs)
    res = None
    for attempt in range(3):
        try:
            res = bass_utils.run_bass_kernel_spmd(
                nc, in_maps, core_ids=list(range(N_CORES)))
            break
        except Exception:
            # transient NRT_EXEC_UNIT_UNRECOVERABLE device wedges clear on
            # retry
            if attempt == 2:
                raise
    out = np.empty((B, DOUT), np.float32)
    for c in range(N_CORES):
        out[c * R:(c + 1) * R, :] = res.results[c]["outT"].T
    return out


if __name__ == "__main__":
    rng = np.random.default_rng(0)
    demo = {
        "x": rng.standard_normal((B, DIN)).astype(np.float32),
        "embed_w": (rng.standard_normal((DH, DIN)) * 0.02).astype(np.float32),
        "embed_b": np.zeros(DH, np.float32),
        "W1": (rng.standard_normal((DFF, DH)) * 0.02).astype(np.float32),
        "b1": np.zeros(DFF, np.float32),
        "W2": (rng.standard_normal((DH, DFF)) * 0.02).astype(np.float32),
        "b2": np.zeros(DH, np.float32),
        "ln_g": np.ones(DH, np.float32),
        "ln_b": np.zeros(DH, np.float32),
        "head_w": (rng.standard_normal((DOUT, DH)) * 0.02).astype(np.float32),
        "head_b": np.zeros(DOUT, np.float32),
    }
    out = kernel(**demo)
    print("out", out.shape, out.dtype, float(np.abs(out).max()))



# revision 8
# speedup vs baseline: 1.9129x; 1.9129x over previous
"""Bass/Tile TRN2 kernel for nn_DimensionScaledEqProp.

Data-parallel over batch: x rows sharded across 8 NeuronCores, weights
replicated. Per-core state stays resident in SBUF across the 30
sequential steps.

fp8e4 (e4m3) matmuls with DoubleRow perf mode for the two big per-step
GEMMs (2 k-tiles per instruction at 0.5 cycles/row = 2x fp16 PE
throughput). Weights pre-scaled on host (x64 for W1, x256*gamma for W2)
to keep fp8 operands out of the subnormal range; the 1/64 folds into the
tanh activation scale and the 1/256 folds into a growing-accumulator
state representation U_t = h_t/(1-gamma)^t, which keeps the state update
a single scalar_tensor_tensor (LayerNorm is scale-invariant, so only
per-step python constants change). Embed / x_emb seed / head matmuls
stay fp16.

Self-contained: hardcodes shapes; host side does sharding, spectral-norm
sigma (tiny: 60 matvecs), weight folding/quantization, and the final
gather/transpose.
"""
import sys
import numpy as np

for _p in ("/opt/trn_rl_repo", "/root/.axon_site/_ro/trn_rl_repo"):
    if _p not in sys.path:
        sys.path.append(_p)

B, DIN, DH, DOUT = 4096, 512, 1024, 256
DFF = 4 * DH
STEPS = 30
N_CORES = 8
R = B // N_CORES  # rows per core = 512
GAMMA = 0.5 * min(1.0, float(np.sqrt(64.0 / DIN)))
OMG = 1.0 - GAMMA
LN_EPS = 1e-5
S1 = 64.0    # host pre-scale on W1eff (fp8 range)
S2 = 256.0   # host pre-scale on gamma*W2n (fp8 range)

KD = DH // 128    # 8  k-tiles over DH
FD = DFF // 128   # 32 f-tiles over DFF
RD = R // 128     # 4  row-tiles per core
ID = DIN // 128   # 4  k-tiles over DIN
OD = DOUT // 128  # 2  out-tiles over DOUT
NH = DH // 512    # 2  psum halves over DH
JK = KD // 2      # 4  DoubleRow pairs over DH
JF = FD // 2      # 16 DoubleRow pairs over DFF

_CACHE = {}


def _build_program(steps: int):
    import concourse.bass as bass
    import concourse.bacc as bacc
    import concourse.mybir as mybir
    from concourse import tile, masks

    f8 = mybir.dt.float8e4
    f16 = mybir.dt.float16
    f32 = mybir.dt.float32
    AF = mybir.ActivationFunctionType
    OP = mybir.AluOpType
    DR = mybir.MatmulPerfMode.DoubleRow

    nc = bacc.Bacc("TRN2", target_bir_lowering=False, debug=False,
                   enable_asserts=True, num_devices=N_CORES)

    xT_d = nc.dram_tensor("xT", [DIN, R], f16, kind="ExternalInput")
    ewT_d = nc.dram_tensor("ewT", [DIN, DH], f16, kind="ExternalInput")
    w1t_d = nc.dram_tensor("w1t", [DH, DFF], f8, kind="ExternalInput")
    b1e_d = nc.dram_tensor("b1e", [DFF, 1], f32, kind="ExternalInput")
    w2t_d = nc.dram_tensor("w2t", [DFF, DH], f8, kind="ExternalInput")
    vb1_d = nc.dram_tensor("vb1", [1, DH], f32, kind="ExternalInput")
    vb2_d = nc.dram_tensor("vb2", [1, DH], f32, kind="ExternalInput")
    hwt_d = nc.dram_tensor("hwt", [DH, DOUT], f16, kind="ExternalInput")
    hb_d = nc.dram_tensor("hb", [DOUT, 1], f32, kind="ExternalInput")
    outT_d = nc.dram_tensor("outT", [DOUT, R], f32, kind="ExternalOutput")

    # per-step python constants for the growing-accumulator state
    # U_s = h_s / OMG^s;  U_{s+1} = U_s + s_coef[s] * p2
    s_coef = [1.0 / (S2 * OMG ** (s + 1)) for s in range(steps)]
    # normalize of U_{s+1} (end of step s) divides by sqrt(var_U + eps_s)
    # where eps_s = LN_EPS / OMG^(2(s+1))
    eps_s = [LN_EPS / OMG ** (2 * (s + 1)) for s in range(steps)]
    C_FIN = OMG ** steps

    with tile.TileContext(nc) as tc:
        with (
            tc.tile_pool(name="wp", bufs=1) as wp,
            tc.tile_pool(name="sp", bufs=1) as sp,
            tc.tile_pool(name="wk", bufs=2) as wk,
            tc.tile_pool(name="stp", bufs=2) as stp,
            tc.tile_pool(name="pst", bufs=4, space="PSUM") as pst,
            tc.tile_pool(name="ps1", bufs=2, space="PSUM") as ps1p,
            tc.tile_pool(name="ps2", bufs=2, space="PSUM") as ps2p,
        ):
            # ---- persistent weights / constants ----
            # DoubleRow pair layouts: free dim = [pair_slot (2), inner]
            w1p = [wp.tile([128, 2 * DFF], f8, name=f"w1p_{j}")
                   for j in range(JK)]
            w2p = [wp.tile([128, 2 * DH], f8, name=f"w2p_{j}")
                   for j in range(JF)]
            hwt = [wp.tile([128, DOUT], f16, name=f"hwt_{k}") for k in range(KD)]
            b1s = wp.tile([128, FD], f32, name="b1s")
            hbs = wp.tile([128, OD], f32, name="hbs")
            ident = wp.tile([128, 128], f16, name="ident")

            # ---- persistent state ----
            h = [sp.tile([128, DH], f32, name=f"h_{r}") for r in range(RD)]
            xeg = [sp.tile([128, DH], f16, name=f"xeg_{r}") for r in range(RD)]
            # hnTp[j]: normalized-state transpose, DoubleRow pair layout
            # [128, 2*R]; slot i holds k-tile 2j+i
            hnTp = [sp.tile([128, 2 * R], f8, name=f"hnTp_{j}")
                    for j in range(JK)]
            # hidTp[j]: tanh output transpose pair layout; slot i = f-tile 2j+i
            hidTp = [sp.tile([128, 2 * R], f8, name=f"hidTp_{j}")
                     for j in range(JF)]
            # fp16 transpose of final h for the head
            hnT16 = [sp.tile([128, R], f16, name=f"hnT16_{k}")
                     for k in range(KD)]

            masks.make_identity(nc, ident[:])

            def w1v(j, f):
                # stationary [128, 2, 128] for mm1 pair j, f-chunk f
                return w1p[j][:].rearrange(
                    "p (two F) -> p two F", two=2)[:, :, f * 128:(f + 1) * 128]

            def w2v(j, half):
                return w2p[j][:].rearrange(
                    "p (two d) -> p two d", two=2)[:, :, half * 512:(half + 1) * 512]

            def hnv(j):
                return hnTp[j][:].rearrange("p (two r) -> p two r", two=2)

            def hidv(j, r):
                return hidTp[j][:].rearrange(
                    "p (two r) -> p two r", two=2)[:, :, r * 128:(r + 1) * 128]

            # ---- embed (transient pool, released before the step loop) ----
            with tc.tile_pool(name="ep", bufs=1) as ep:
                xts = [ep.tile([128, R], f16, name=f"xts_{i}")
                       for i in range(ID)]
                ewt = [ep.tile([128, DH], f16, name=f"ewt_{i}")
                       for i in range(ID)]
                bc1 = ep.tile([128, DH], f32, name="bc1")
                bc2 = ep.tile([128, DH], f32, name="bc2")
                for i in range(ID):
                    nc.sync.dma_start(
                        xts[i][:], xT_d.ap()[i * 128:(i + 1) * 128, :])
                    nc.sync.dma_start(
                        ewt[i][:], ewT_d.ap()[i * 128:(i + 1) * 128, :])
                nc.sync.dma_start(bc1[0:1, :], vb1_d.ap())
                nc.sync.dma_start(bc2[0:1, :], vb2_d.ap())
                nc.gpsimd.partition_broadcast(bc1[:], bc1[0:1, :])
                nc.gpsimd.partition_broadcast(bc2[:], bc2[0:1, :])

                # weight loads AFTER embed inputs: embed matmuls start
                # immediately; w1 streams in behind them
                for j in range(JK):
                    nc.sync.dma_start(
                        w1p[j][:, 0:DFF],
                        w1t_d.ap()[(2 * j) * 128:(2 * j + 1) * 128, :])
                    nc.sync.dma_start(
                        w1p[j][:, DFF:2 * DFF],
                        w1t_d.ap()[(2 * j + 1) * 128:(2 * j + 2) * 128, :])
                nc.sync.dma_start(
                    b1s[:], b1e_d.ap().rearrange("(f p) o -> p (f o)", p=128))
                nc.sync.dma_start(
                    hbs[:], hb_d.ap().rearrange("(t p) o -> p (t o)", p=128))

                # h0 = x @ ewT + embed_b ; xeg = S2*g*(h0 + b2) (f16)
                # (vb2 already holds S2*g*b2)
                for r in range(RD):
                    for half in range(NH):
                        sl = slice(half * 512, (half + 1) * 512)
                        pe = ps1p.tile([128, 512], f32, tag="ps1", name="pe")
                        for i in range(ID):
                            nc.tensor.matmul(
                                pe[:], xts[i][:, r * 128:(r + 1) * 128],
                                ewt[i][:, sl],
                                start=(i == 0), stop=(i == ID - 1))
                        nc.vector.tensor_tensor(
                            h[r][:, sl], pe[:], bc1[:, sl], op=OP.add)
                        nc.vector.scalar_tensor_tensor(
                            xeg[r][:, sl], h[r][:, sl], S2 * GAMMA,
                            bc2[:, sl], op0=OP.mult, op1=OP.add)

            # ---- initial LN stats on h0 (ACT sqrt once; hides table load) ----
            mv0 = stp.tile([128, RD * 2], f32, tag="mv", name="mv_init")
            for r in range(RD):
                st6 = stp.tile([128, 12], f32, tag="st6", name=f"st6_i_{r}")
                for c in range(2):
                    nc.vector.bn_stats(
                        st6[:, c * 6:(c + 1) * 6],
                        h[r][:, c * 512:(c + 1) * 512])
                nc.vector.bn_aggr(
                    mv0[:].rearrange("p (r x) -> p r x", x=2)[:, r], st6[:])
            ve0 = stp.tile([128, RD], f32, tag="ve", name="ve_init")
            nc.vector.tensor_scalar(
                ve0[:], mv0[:].rearrange("p (r x) -> p x r", x=2)[:, 1],
                LN_EPS, None, op0=OP.add)
            rv0 = stp.tile([128, RD], f32, tag="rv", name="rv_init")
            nc.vector.reciprocal(rv0[:], ve0[:])
            rs_prev = stp.tile([128, RD], f32, tag="rs", name="rs_init")
            nc.scalar.activation(rs_prev[:], rv0[:], AF.Sqrt)

            def rstd_newton(y_out, y_seed, var_ap, tag_sfx, n_iter=2,
                            seed_c=1.0, eps=LN_EPS):
                """y_out[128,1] = 1/sqrt(var+eps) via Newton from seed.

                seed_c: constant rescale folded onto the seed (the seed is
                seed_c * y_seed); eps may differ per step (U-scaling).
                """
                hv = stp.tile([128, 1], f32, tag="hv", name=f"hv_{tag_sfx}")
                nc.vector.tensor_scalar(
                    hv[:], var_ap, -0.5, -0.5 * eps,
                    op0=OP.mult, op1=OP.add)
                if seed_c != 1.0:
                    # hv scaled by c^2 for the first iteration only; the
                    # c itself folds into that iteration's final multiply
                    hvc = stp.tile([128, 1], f32, tag="hv",
                                   name=f"hvc_{tag_sfx}")
                    nc.vector.tensor_scalar(
                        hvc[:], var_ap, -0.5 * seed_c * seed_c,
                        -0.5 * eps * seed_c * seed_c, op0=OP.mult, op1=OP.add)
                else:
                    hvc = hv
                y = y_seed
                cur_c = seed_c
                for it in range(n_iter):
                    a = stp.tile([128, 1], f32, tag="nwa",
                                 name=f"nwa_{tag_sfx}_{it}")
                    nc.vector.tensor_tensor(a[:], y, y, op=OP.mult)
                    nc.vector.tensor_scalar(
                        a[:], a[:], (hvc if it == 0 else hv)[:], 1.5,
                        op0=OP.mult, op1=OP.add)
                    dst = y_out if it == n_iter - 1 else stp.tile(
                        [128, 1], f32, tag="nwy",
                        name=f"nwy_{tag_sfx}_{it}")[:]
                    if cur_c != 1.0:
                        nc.vector.scalar_tensor_tensor(
                            dst, y, cur_c, a[:], op0=OP.mult, op1=OP.mult)
                    else:
                        nc.vector.tensor_tensor(dst, y, a[:], op=OP.mult)
                    y = dst
                    cur_c = 1.0

            # normalize h0 -> hnTp for step 0 (one Newton polish on seed)
            rs_fix = stp.tile([128, RD], f32, tag="rsf", name="rs_fix")
            for r in range(RD):
                rstd_newton(rs_fix[:, r:r + 1], rs_prev[:, r:r + 1],
                            mv0[:, 2 * r + 1:2 * r + 2], f"i{r}", n_iter=1)
            rs_p = rs_fix

            def normalize(r, mean_ap, rs_col, sfx):
                nmu = stp.tile([128, 1], f32, tag="nmu", name=f"nmu_{sfx}")
                nc.vector.scalar_tensor_tensor(
                    nmu[:], mean_ap, -1.0, rs_col,
                    op0=OP.mult, op1=OP.mult)
                hn16 = wk.tile([128, DH], f16, tag=f"hn16_{r}",
                               name=f"hn16_{sfx}", bufs=1)
                # two half-width ops so transposes of the low half can
                # start before the high half is normalized
                for half in range(NH):
                    sl = slice(half * 512, (half + 1) * 512)
                    nc.vector.tensor_scalar(
                        hn16[:, sl], h[r][:, sl], rs_col, nmu[:],
                        op0=OP.mult, op1=OP.add)
                return hn16

            def transposes(r, hn16, sfx):
                # fp16 PE transpose (fp8 transpose needs element-step-2
                # output APs); the PSUM->SBUF copy does the fp8 cast
                for k in range(KD):
                    tp = pst.tile([128, 128], f16, tag="tp",
                                  name=f"tp_{sfx}_{k}")
                    nc.tensor.transpose(
                        tp[:], hn16[:, k * 128:(k + 1) * 128], ident[:])
                    dst = hnTp[k // 2][:, (k % 2) * R + r * 128:
                                       (k % 2) * R + (r + 1) * 128]
                    if k % 2 == 0:
                        nc.vector.tensor_copy(dst, tp[:])
                    else:
                        nc.scalar.copy(dst, tp[:])

            for r in range(RD):
                hn = normalize(r, mv0[:, 2 * r:2 * r + 1],
                               rs_p[:, r:r + 1], f"s0_{r}")
                transposes(r, hn, f"s0_{r}")

            for s in range(steps):
                last = (s == steps - 1)
                # hidT = tanh((W1q' @ hnT)/S1 + b1)  [fp8 DoubleRow]
                for f in range(FD):
                    p1 = ps1p.tile([128, 512], f32, tag="ps1",
                                   name=f"p1_{s}_{f}")
                    for j in range(JK):
                        nc.tensor.matmul(
                            p1[:], w1v(j, f), hnv(j),
                            start=(j == 0), stop=(j == JK - 1),
                            perf_mode=DR)
                    nc.scalar.activation(
                        hidTp[f // 2][:, (f % 2) * R:(f % 2 + 1) * R],
                        p1[:], AF.Tanh, bias=b1s[:, f:f + 1], scale=1.0 / S1)

                if s == 0:
                    # w2/hwt loads deferred past step-0 mm1 so w1 gets
                    # full DMA bandwidth at startup; w2 arrives during
                    # mm1 execution, well before mm2 needs it
                    for j_ in range(JF):
                        nc.sync.dma_start(
                            w2p[j_][:, 0:DH],
                            w2t_d.ap()[(2 * j_) * 128:(2 * j_ + 1) * 128, :])
                        nc.sync.dma_start(
                            w2p[j_][:, DH:2 * DH],
                            w2t_d.ap()[(2 * j_ + 1) * 128:
                                       (2 * j_ + 2) * 128, :])
                    for k_ in range(KD):
                        nc.sync.dma_start(
                            hwt[k_][:],
                            hwt_d.ap()[k_ * 128:(k_ + 1) * 128, :])

                # per row-tile: matmul2 (+xeg seeded in PSUM), update,
                # stats, rstd, normalize, transpose -- interleaved so PE
                # never idles at the step boundary.
                mv = stp.tile([128, RD * 2], f32, tag="mv", name=f"mv_{s}")
                mvv = mv[:].rearrange("p (r x) -> p r x", x=2)
                rs = stp.tile([128, RD], f32, tag="rs", name=f"rs_{s}")
                hns = {}
                for r in range(RD):
                    st6 = stp.tile([128, 12], f32, tag="st6",
                                   name=f"st6_{s}_{r}")
                    for half in range(NH):
                        sl = slice(half * 512, (half + 1) * 512)
                        p2 = ps2p.tile([128, 512], f32, tag="ps2",
                                       name=f"p2_{s}_{r}_{half}")
                        nc.tensor.matmul(
                            p2[:], ident[:], xeg[r][:, sl],
                            start=True, stop=False)
                        for j in range(JF):
                            nc.tensor.matmul(
                                p2[:], hidv(j, r), w2v(j, half),
                                start=False, stop=(j == JF - 1),
                                perf_mode=DR)
                        # U += s_coef * p2
                        nc.vector.scalar_tensor_tensor(
                            h[r][:, sl], p2[:], s_coef[s], h[r][:, sl],
                            op0=OP.mult, op1=OP.add)
                        if not last:
                            nc.vector.bn_stats(
                                st6[:, half * 6:(half + 1) * 6],
                                h[r][:, sl])
                    if last:
                        # head prep inline: fold OMG^steps into the final
                        # fp16 cast so its transposes overlap the
                        # remaining matmul2 groups
                        hc16 = wk.tile([128, DH], f16, tag=f"hn16_{r}",
                                       name=f"hc16_{r}", bufs=1)
                        nc.vector.tensor_scalar(
                            hc16[:], h[r][:], C_FIN, None, op0=OP.mult)
                        hns[r] = hc16
                        continue
                    nc.vector.bn_aggr(mvv[:, r], st6[:])
                    rstd_newton(rs[:, r:r + 1], rs_p[:, r:r + 1],
                                mv[:, 2 * r + 1:2 * r + 2], f"{s}_{r}",
                                seed_c=OMG, eps=eps_s[s])
                    hns[r] = normalize(r, mv[:, 2 * r:2 * r + 1],
                                       rs[:, r:r + 1], f"{s}_{r}")
                # transposes LAST: PE has cover work while the final
                # row-tile's DVE chain drains, so it never idles
                if last:
                    for r in range(RD):
                        for k in range(KD):
                            tp = pst.tile([128, 128], f16, tag="tp",
                                          name=f"tpf_{r}_{k}")
                            nc.tensor.transpose(
                                tp[:], hns[r][:, k * 128:(k + 1) * 128],
                                ident[:])
                            if k % 2 == 0:
                                nc.vector.tensor_copy(
                                    hnT16[k][:, r * 128:(r + 1) * 128], tp[:])
                            else:
                                nc.scalar.copy(
                                    hnT16[k][:, r * 128:(r + 1) * 128], tp[:])
                else:
                    for r in range(RD):
                        transposes(r, hns[r], f"{s}_{r}")
                rs_p = rs

            # ---- head: outT = head_w @ h.T + head_b ----
            for ot in range(OD):
                po = ps1p.tile([128, 512], f32, tag="ps1", name=f"po_{ot}")
                for k in range(KD):
                    nc.tensor.matmul(
                        po[:], hwt[k][:, ot * 128:(ot + 1) * 128],
                        hnT16[k][:],
                        start=(k == 0), stop=(k == KD - 1))
                osb = wk.tile([128, 512], f32, tag="osb",
                              name=f"osb_{ot}", bufs=1)
                nc.scalar.activation(
                    osb[:], po[:], AF.Identity, bias=hbs[:, ot:ot + 1])
                nc.sync.dma_start(
                    outT_d.ap()[ot * 128:(ot + 1) * 128, :], osb[:])

    nc.compile()
    return nc


def _get_compiled(steps: int):
    key = ("prog", steps)
    if key not in _CACHE:
        from concourse.bass_interp import get_hw_module
        nc = _build_program(steps)
        nc.m = get_hw_module(nc.m)
        _CACHE[key] = nc
    return _CACHE[key]


def _spectral_sigma(W: np.ndarray) -> float:
    W = W.astype(np.float64)
    v = np.full(W.shape[1], 1.0 / np.sqrt(W.shape[1]))
    u = W @ v
    u = u / (np.linalg.norm(u) + 1e-12)
    for _ in range(15):
        u = W @ v
        u = u / (np.linalg.norm(u) + 1e-12)
        v = W.T @ u
        v = v / (np.linalg.norm(v) + 1e-12)
    return float(u @ (W @ v))


def _prep_host(inputs: dict) -> tuple[dict, list]:
    import ml_dtypes
    f8np = ml_dtypes.float8_e4m3

    f = {k: np.asarray(v, dtype=np.float32) for k, v in inputs.items()}
    x, ew, eb = f["x"], f["embed_w"], f["embed_b"]
    W1, b1, W2, b2 = f["W1"], f["b1"], f["W2"], f["b2"]
    ln_g, ln_b = f["ln_g"], f["ln_b"]
    hw_, hb = f["head_w"], f["head_b"]

    s1 = _spectral_sigma(W1)
    s2 = _spectral_sigma(W2)
    W1n = (W1.astype(np.float64) / s1)
    W2n = (W2.astype(np.float64) / s2)
    # fold ln gain into W1, ln bias into b1
    W1eff = W1n * ln_g.astype(np.float64)[None, :]
    b1eff = (b1.astype(np.float64) + W1n @ ln_b.astype(np.float64))
    W2eff = GAMMA * W2n

    shared = {
        "ewT": np.ascontiguousarray(ew.T).astype(np.float16),
        "w1t": np.ascontiguousarray((S1 * W1eff).T).astype(f8np),
        "b1e": b1eff.astype(np.float32).reshape(DFF, 1),
        "w2t": np.ascontiguousarray((S2 * W2eff).T).astype(f8np),
        "vb1": eb.reshape(1, DH).astype(np.float32),
        "vb2": (S2 * GAMMA * b2).reshape(1, DH).astype(np.float32),
        "hwt": np.ascontiguousarray(hw_.T).astype(np.float16),
        "hb": hb.reshape(DOUT, 1).astype(np.float32),
    }
    in_maps = []
    for c in range(N_CORES):
        shard = x[c * R:(c + 1) * R, :]
        m = dict(shared)
        m["xT"] = np.ascontiguousarray(shard.T).astype(np.float16)
        in_maps.append(m)
    return shared, in_maps


def kernel(**inputs) -> np.ndarray:
    from concourse import bass_utils
    nc = _get_compiled(STEPS)
    _, in_maps = _prep_host(inputs)
    res = None
    for attempt in range(3):
        try:
            res = bass_utils.run_bass_kernel_spmd(
                nc, in_maps, core_ids=list(range(N_CORES)))
            break
        except Exception:
            # transient NRT_EXEC_UNIT_UNRECOVERABLE device wedges clear on
            # retry
            if attempt == 2:
                raise
    out = np.empty((B, DOUT), np.float32)
    for c in range(N_CORES):
        out[c * R:(c + 1) * R, :] = res.results[c]["outT"].T
    return out


if __name__ == "__main__":
    rng = np.random.default_rng(0)
    demo = {
        "x": rng.standard_normal((B, DIN)).astype(np.float32),
        "embed_w": (rng.standard_normal((DH, DIN)) * 0.02).astype(np.float32),
        "embed_b": np.zeros(DH, np.float32),
        "W1": (rng.standard_normal((DFF, DH)) * 0.02).astype(np.float32),
        "b1": np.zeros(DFF, np.float32),
        "W2": (rng.standard_normal((DH, DFF)) * 0.02).astype(np.float32),
        "b2": np.zeros(DH, np.float32),
        "ln_g": np.ones(DH, np.float32),
        "ln_b": np.zeros(DH, np.float32),
        "head_w": (rng.standard_normal((DOUT, DH)) * 0.02).astype(np.float32),
        "head_b": np.zeros(DOUT, np.float32),
    }
    out = kernel(**demo)
    print("out", out.shape, out.dtype, float(np.abs(out).max()))
